# revision 1
# baseline (speedup 1.0000x reference)
"""Trainium2 Bass kernel for nn_Net_66408784331557 (dense MLP with sync-BN).

Reference computation:
    h = BN_train(x; gamma_in, beta_in)            # x: [65536, 2048]
    h = relu(h @ W_in.T + b_in)                   # -> [65536, 75]
    12x: h = relu(BN_train(h; g_l, b_l) @ W_l.T + bias_l)
    out = h @ W_out.T + b_out                     # -> [65536, 1]

Strategy: data-parallel over the batch across 8 NeuronCores (8192 rows each).
BatchNorm is algebraically folded into the following Linear layer:
    BN(x) @ W.T = x @ (W * s).T + (beta - mu*s) @ W.T,  s = gamma*rsqrt(var+eps)
so each layer needs only the global per-feature (sum, sumsq) -> one tiny
AllReduce per BN, then a weight fold, then a plain matmul + fused ReLU.

Matmuls run in float32r (fp32 storage, ~11-bit-mantissa TensorEngine mode)
for accuracy; layer-1 stats are computed in bf16 (PE ones-matmul colsums).

Layer 1 is two passes over x (stats pass, then matmul pass); the 12 middle
layers keep h resident in SBUF in [feature, batch] layout.
"""

import sys
import functools

import numpy as np

for _p in ("/opt/trn_rl_repo",):
    if _p not in sys.path:
        sys.path.insert(0, _p)

import ml_dtypes

N_CORES = 8
B = 65536
D = 2048
H = 75
L = 12
N_OUT = 1
EPS = 1e-5

BF16 = ml_dtypes.bfloat16

WARM_DUMMIES = True   # tiny bf16 matmuls interleaved to keep PE HAM warm
F32R_TRANSPOSE = False  # f32r-mode transpose rejected by BIR verifier (unrounded in)


def build_program(n_cores=N_CORES, b_local=B // N_CORES, d=D, h=H, n_layers=L,
                  debug=False):
    """Builds the SPMD Bass/Tile program (identical on every core)."""
    import concourse.bass as bass
    import concourse.mybir as mybir
    import concourse.tile as tile
    from concourse import bacc

    f32 = mybir.dt.float32
    f32r = mybir.dt.float32r
    bf16 = mybir.dt.bfloat16
    AF = mybir.ActivationFunctionType
    ALU = mybir.AluOpType

    QD = d // 128          # feature chunks of 128
    CC = d // 512          # colsum chunks of 512
    NT = b_local // 128    # pass-1 tiles (128 rows each)
    BCN = b_local // 512   # batch chunks of 512
    B_TOT = n_cores * b_local
    tdt = f32r if F32R_TRANSPOSE else f32

    nc = bacc.Bacc("TRN2", target_bir_lowering=False, debug=debug,
                   enable_asserts=True, num_devices=n_cores)

    # ---- I/O ----
    x_d = nc.dram_tensor("x", [b_local, d], f32, kind="ExternalInput").ap()
    wint_d = nc.dram_tensor("wint", [d, h], f32, kind="ExternalInput").ap()
    bin_d = nc.dram_tensor("bin", [h, 1], f32, kind="ExternalInput").ap()
    grow_d = nc.dram_tensor("grow", [1, d], f32, kind="ExternalInput").ap()
    brow_d = nc.dram_tensor("brow", [1, d], f32, kind="ExternalInput").ap()
    midwt_d = nc.dram_tensor("midwt", [n_layers, h, h], f32, kind="ExternalInput").ap()
    midg_d = nc.dram_tensor("midg", [h, n_layers], f32, kind="ExternalInput").ap()
    midbeta_d = nc.dram_tensor("midbeta", [h, n_layers], f32, kind="ExternalInput").ap()
    midbias_d = nc.dram_tensor("midbias", [h, n_layers], f32, kind="ExternalInput").ap()
    woutt_d = nc.dram_tensor("woutt", [h, N_OUT], f32, kind="ExternalInput").ap()
    bout_d = nc.dram_tensor("bout", [1, 1], f32, kind="ExternalInput").ap()
    identf_d = nc.dram_tensor("identf", [128, 128], f32, kind="ExternalInput").ap()
    identf2_d = nc.dram_tensor("identf2", [2, 2], f32, kind="ExternalInput").ap()
    onesbf_d = nc.dram_tensor("onesbf", [128, 1], bf16, kind="ExternalInput").ap()
    out_d = nc.dram_tensor("out", [b_local, N_OUT], f32, kind="ExternalOutput").ap()

    rg = [list(range(n_cores))]

    with tile.TileContext(nc) as tc:
        with tc.tile_pool(name="const", bufs=1) as cp, \
             tc.tile_pool(name="drp", bufs=1, space="DRAM") as drp:

            # ---- constants into SBUF ----
            wint_sb = cp.tile([128, QD, h], f32)
            nc.sync.dma_start(wint_sb, wint_d.rearrange("(q p) h -> p q h", p=128))
            bin_sb = cp.tile([h, 1], f32)
            nc.sync.dma_start(bin_sb, bin_d)
            grow_sb = cp.tile([1, d], f32)
            nc.sync.dma_start(grow_sb, grow_d)
            brow_sb = cp.tile([1, d], f32)
            nc.sync.dma_start(brow_sb, brow_d)
            midwt_sb = cp.tile([h, n_layers, h], f32)
            nc.sync.dma_start(midwt_sb, midwt_d.rearrange("l k o -> k l o"))
            midg_sb = cp.tile([h, n_layers], f32)
            nc.sync.dma_start(midg_sb, midg_d)
            midbeta_sb = cp.tile([h, n_layers], f32)
            nc.sync.dma_start(midbeta_sb, midbeta_d)
            midbias_sb = cp.tile([h, n_layers], f32)
            nc.sync.dma_start(midbias_sb, midbias_d)
            woutt_sb = cp.tile([h, N_OUT], f32)
            nc.sync.dma_start(woutt_sb, woutt_d)
            bout_sb = cp.tile([1, 1], f32)
            nc.sync.dma_start(bout_sb, bout_d)
            identf = cp.tile([128, 128], f32)
            nc.sync.dma_start(identf, identf_d)
            identf2 = cp.tile([2, 2], f32)
            nc.sync.dma_start(identf2, identf2_d)
            onesbf = cp.tile([128, 1], bf16)
            nc.sync.dma_start(onesbf, onesbf_d)

            ident_t = identf.bitcast(f32r) if F32R_TRANSPOSE else identf

            def dummy_warm(pool, name):
                # tiny bf16 matmul; bf16 activity keeps the PE HAM un-throttled
                if not WARM_DUMMIES:
                    return
                pd = pool.tile([1, 1], f32, tag="pdum", name=name)
                nc.tensor.matmul(pd, onesbf, onesbf, skip_group_check=True)

            # fold-phase scratch pool (released before pass 2)
            sp = tc.alloc_tile_pool(name="fold", bufs=1)

            # =========== PASS 1: per-feature sum / sumsq of x ===========
            with tc.tile_pool(name="p1", bufs=6) as p1, \
                 tc.tile_pool(name="p1ps", bufs=1, space="PSUM") as p1ps:
                ps_sum = [p1ps.tile([1, 512], f32, name=f"ps_sum{c}") for c in range(CC)]
                ps_sq = [p1ps.tile([1, 512], f32, name=f"ps_sq{c}") for c in range(CC)]
                for i in range(NT):
                    xf = p1.tile([128, d], f32, tag="xf", name=f"xf{i}")
                    nc.sync.dma_start(xf, x_d[i * 128:(i + 1) * 128, :])
                    xt = p1.tile([128, d], bf16, tag="xt", name=f"xt{i}", bufs=4)
                    nc.vector.tensor_copy(xt, xf)  # fp32 -> bf16 (2x mode)
                    xsq = p1.tile([128, d], bf16, tag="xsq", name=f"xsq{i}", bufs=4)
                    nc.scalar.square(xsq, xt)
                    for c in range(CC):
                        nc.tensor.matmul(ps_sum[c], onesbf, xt[:, c * 512:(c + 1) * 512],
                                         start=(i == 0), stop=(i == NT - 1),
                                         skip_group_check=True)
                    for c in range(CC):
                        nc.tensor.matmul(ps_sq[c], onesbf, xsq[:, c * 512:(c + 1) * 512],
                                         start=(i == 0), stop=(i == NT - 1),
                                         skip_group_check=True)

                stats_row = sp.tile([1, 2 * d], f32)
                for c in range(CC):
                    nc.vector.tensor_copy(stats_row[:, c * 512:(c + 1) * 512], ps_sum[c])
                for c in range(CC):
                    nc.vector.tensor_copy(stats_row[:, d + c * 512:d + (c + 1) * 512], ps_sq[c])

            # ---- AllReduce of [sum | sumsq] (scalar-engine DMAs keep the
            # sync-engine stream free for pass-2 prefetch) ----
            st1i = drp.tile([1, 2 * d], f32)
            st1o = drp.tile([1, 2 * d], f32)
            nc.scalar.dma_start(st1i, stats_row)
            nc.gpsimd.collective_compute(
                "AllReduce", mybir.AluOpType.add, replica_groups=rg,
                ins=[st1i.opt()], outs=[st1o.opt()])
            g_row = stats_row  # reuse the SBUF row for the reduced result
            nc.scalar.dma_start(g_row, st1o)

            # ---- stats -> (s, t) rows; Newton-polished rsqrt ----
            mu = sp.tile([1, d], f32)
            tmp1 = sp.tile([1, d], f32)
            tmp2 = sp.tile([1, d], f32)
            tmp3 = sp.tile([1, d], f32)
            nc.vector.tensor_scalar_mul(mu, g_row[:, 0:d], 1.0 / B_TOT)
            vep = g_row[:, d:2 * d]
            nc.vector.tensor_scalar(out=vep, in0=vep,
                                    scalar1=1.0 / B_TOT, scalar2=float(EPS),
                                    op0=ALU.mult, op1=ALU.add)  # E[x^2]+eps
            nc.vector.tensor_tensor(out=tmp1, in0=mu, in1=mu, op=ALU.mult)
            nc.vector.tensor_tensor(out=vep, in0=vep, in1=tmp1,
                                    op=ALU.subtract)  # var+eps
            nc.scalar.activation(tmp2, vep, AF.Sqrt)
            nc.vector.reciprocal(tmp3, tmp2)
            # one Newton step: r = r0*(1.5 - 0.5*vep*r0^2)
            nc.vector.tensor_tensor(out=tmp1, in0=tmp3, in1=tmp3, op=ALU.mult)
            nc.vector.tensor_tensor(out=tmp2, in0=vep, in1=tmp1, op=ALU.mult)
            nc.vector.tensor_scalar(out=tmp2, in0=tmp2, scalar1=-0.5, scalar2=1.5,
                                    op0=ALU.mult, op1=ALU.add)
            nc.vector.tensor_tensor(out=tmp1, in0=tmp3, in1=tmp2, op=ALU.mult)  # r
            s_row = tmp3
            nc.vector.tensor_tensor(out=s_row, in0=tmp1, in1=grow_sb, op=ALU.mult)
            nc.vector.tensor_tensor(out=tmp2, in0=mu, in1=s_row, op=ALU.mult)
            t_row = tmp1
            nc.vector.tensor_tensor(out=t_row, in0=brow_sb, in1=tmp2,
                                    op=ALU.subtract)  # t = beta - mu*s

            # transpose s,t rows into per-partition layout [128, QD]
            with tc.tile_pool(name="stps", bufs=1, space="PSUM") as stps:
                ps_st = stps.tile([128, QD, 2], f32)
                for q in range(QD):
                    nc.tensor.matmul(ps_st[:, q, 0:1],
                                     s_row[:, q * 128:(q + 1) * 128],
                                     identf2[0:1, 0:1], is_transpose=True,
                                     skip_group_check=True)
                    nc.tensor.matmul(ps_st[:, q, 1:2],
                                     t_row[:, q * 128:(q + 1) * 128],
                                     identf2[0:1, 0:1], is_transpose=True,
                                     skip_group_check=True)
                sT = sp.tile([128, QD], f32)
                tT = sp.tile([128, QD], f32)
                nc.vector.tensor_copy(sT, ps_st[:, :, 0])
                nc.vector.tensor_copy(tT, ps_st[:, :, 1])

            # fold: wfold[:,q,:] = wint[:,q,:] * sT[:,q], bias1 = b_in + W_in @ t
            wfold = cp.tile([128, QD, h], f32r)
            for q in range(QD):
                nc.vector.tensor_scalar_mul(wfold[:, q, :], wint_sb[:, q, :],
                                            sT[:, q:q + 1])
            with tc.tile_pool(name="pbias", bufs=1, space="PSUM") as pbias:
                ps_b1 = pbias.tile([h, 1], f32)
                for q in range(QD):
                    nc.tensor.matmul(ps_b1, wint_sb[:, q, :], tT[:, q:q + 1],
                                     start=(q == 0), stop=(q == QD - 1),
                                     skip_group_check=True)
                bias1 = cp.tile([h, 1], f32)
                nc.vector.tensor_tensor(out=bias1, in0=ps_b1, in1=bin_sb, op=ALU.add)

            sp.release()  # fold scratch freed before pass 2

            # h buffers, [feature, batch] layout, f32r
            hp = tc.alloc_tile_pool(name="hpool", bufs=1)
            h_a = hp.tile([h, b_local], f32r)
            h_b = hp.tile([h, b_local], f32r)
            # bn-stats staging (layer-0 stats are produced inside pass 2)
            bnp = tc.alloc_tile_pool(name="bnp", bufs=2)
            bnst0 = bnp.tile([h, BCN, 6], f32, tag="bnst", name="bnst_l0")

            # =========== PASS 2: h1 = relu(x_norm @ W_in'.T + bias1) ===========
            with tc.tile_pool(name="p2x", bufs=6) as p2x, \
                 tc.tile_pool(name="p2t", bufs=24 if QD == 16 else 3 * QD) as p2t, \
                 tc.tile_pool(name="p2ps", bufs=3, space="PSUM") as p2ps, \
                 tc.tile_pool(name="p2ph", bufs=2, space="PSUM") as p2ph, \
                 tc.tile_pool(name="p2pd", bufs=1, space="PSUM") as p2pd:
                for bc in range(BCN):
                    xbs = []
                    for t4 in range(4):
                        xb = p2x.tile([128, d], f32, tag="x2", name=f"x2_{bc}_{t4}")
                        r0_ = bc * 512 + t4 * 128
                        nc.sync.dma_start(xb, x_d[r0_:r0_ + 128, :])
                        xbs.append(xb)
                    xts = []
                    for q in range(QD):
                        pst = p2ps.tile([128, 512], tdt, tag="pst", name=f"pst{bc}_{q}")
                        for t4 in range(4):
                            nc.tensor.matmul(
                                pst[:, t4 * 128:(t4 + 1) * 128],
                                xbs[t4][:, q * 128:(q + 1) * 128].bitcast(tdt),
                                ident_t, is_transpose=True,
                                skip_group_check=True)
                        xtq = p2t.tile([128, 512], f32r, tag="xT", name=f"xT{bc}_{q}")
                        if q % 2 == 0:
                            nc.vector.tensor_copy(xtq, pst)
                        else:
                            nc.scalar.copy(xtq, pst)
                        xts.append(xtq)
                        if q % 4 == 0:
                            dummy_warm(p2pd, f"dum{bc}_{q}")
                    psh = p2ph.tile([h, 512], f32, tag="psh", name=f"psh{bc}")
                    for q in range(QD):
                        nc.tensor.matmul(psh, wfold[:, q, :], xts[q],
                                         start=(q == 0), stop=(q == QD - 1),
                                         skip_group_check=True)
                    nc.scalar.activation(h_a[:, bc * 512:(bc + 1) * 512], psh,
                                         AF.Relu, bias=bias1[:, 0:1])
                    # layer-0 bn stats ride along with pass-2
                    nc.vector.bn_stats(bnst0[:, bc, :],
                                       h_a.bitcast(f32)[:, bc * 512:(bc + 1) * 512])

            # =========== 12 middle layers ===========
            h_in, h_out = h_a, h_b
            bnst = bnst0
            with tc.tile_pool(name="mid", bufs=2) as mp_, \
                 tc.tile_pool(name="midps", bufs=3, space="PSUM") as mps, \
                 tc.tile_pool(name="midpb", bufs=1, space="PSUM") as mpb, \
                 tc.tile_pool(name="midpd", bufs=1, space="PSUM") as mpd:
                for l in range(n_layers):
                    mv = mp_.tile([h, 2], f32, tag="mv", name=f"mv{l}")
                    nc.vector.bn_aggr(mv, bnst)
                    # payload [mean, var+mean^2] = [mean, E[h^2]]
                    pay = mp_.tile([h, 2], f32, tag="pay", name=f"pay{l}")
                    nc.vector.tensor_copy(pay[:, 0:1], mv[:, 0:1])
                    msq = mp_.tile([h, 1], f32, tag="msq", name=f"msq{l}")
                    nc.vector.tensor_tensor(out=msq, in0=mv[:, 0:1], in1=mv[:, 0:1],
                                            op=ALU.mult)
                    nc.vector.tensor_tensor(out=pay[:, 1:2], in0=mv[:, 1:2], in1=msq,
                                            op=ALU.add)
                    mbi = drp.tile([h, 2], f32, name=f"mbi{l}")
                    mbo = drp.tile([h, 2], f32, name=f"mbo{l}")
                    nc.scalar.dma_start(mbi, pay)
                    nc.gpsimd.collective_compute(
                        "AllReduce", mybir.AluOpType.add, replica_groups=rg,
                        ins=[mbi.opt()], outs=[mbo.opt()])
                    g2 = mp_.tile([h, 2], f32, tag="g2", name=f"g2{l}")
                    nc.scalar.dma_start(g2, mbo)

                    mug = mp_.tile([h, 1], f32, tag="mug", name=f"mug{l}")
                    nc.vector.tensor_scalar_mul(mug, g2[:, 0:1], 1.0 / n_cores)
                    veg = mp_.tile([h, 1], f32, tag="veg", name=f"veg{l}")
                    nc.vector.tensor_scalar(out=veg, in0=g2[:, 1:2],
                                            scalar1=1.0 / n_cores, scalar2=float(EPS),
                                            op0=ALU.mult, op1=ALU.add)
                    musq2 = mp_.tile([h, 1], f32, tag="musq2", name=f"musq2{l}")
                    nc.vector.tensor_tensor(out=musq2, in0=mug, in1=mug, op=ALU.mult)
                    vef = mp_.tile([h, 1], f32, tag="vef", name=f"vef{l}")
                    nc.vector.tensor_tensor(out=vef, in0=veg, in1=musq2, op=ALU.subtract)
                    sd2 = mp_.tile([h, 1], f32, tag="sd2", name=f"sd2{l}")
                    nc.scalar.activation(sd2, vef, AF.Sqrt)
                    rr = mp_.tile([h, 1], f32, tag="rr", name=f"rr{l}")
                    nc.vector.reciprocal(rr, sd2)
                    s2 = mp_.tile([h, 1], f32, tag="s2", name=f"s2{l}")
                    nc.vector.tensor_tensor(out=s2, in0=rr, in1=midg_sb[:, l:l + 1],
                                            op=ALU.mult)
                    mt = mp_.tile([h, 1], f32, tag="mt", name=f"mt{l}")
                    nc.vector.tensor_tensor(out=mt, in0=mug, in1=s2, op=ALU.mult)
                    t2 = mp_.tile([h, 1], f32, tag="t2", name=f"t2{l}")
                    nc.vector.tensor_tensor(out=t2, in0=midbeta_sb[:, l:l + 1], in1=mt,
                                            op=ALU.subtract)
                    wf = mp_.tile([h, h], f32r, tag="wf", name=f"wf{l}")
                    nc.vector.tensor_scalar_mul(wf, midwt_sb[:, l, :], s2)
                    ps_b2 = mpb.tile([h, 1], f32, tag="psb2", name=f"psb2_{l}")
                    nc.tensor.matmul(ps_b2, midwt_sb[:, l, :], t2,
                                     skip_group_check=True)
                    bias2 = mp_.tile([h, 1], f32, tag="bias2", name=f"bias2{l}")
                    nc.vector.tensor_tensor(out=bias2, in0=ps_b2,
                                            in1=midbias_sb[:, l:l + 1], op=ALU.add)

                    bnst = bnp.tile([h, BCN, 6], f32, tag="bnst", name=f"bnst_l{l+1}")
                    h_out_f = h_out.bitcast(f32)
                    for bc in range(BCN):
                        psm = mps.tile([h, 512], f32, tag="psm", name=f"psm{l}_{bc}")
                        if bc % 4 == 0:
                            dummy_warm(mpd, f"mdum{l}_{bc}")
                        nc.tensor.matmul(psm, wf, h_in[:, bc * 512:(bc + 1) * 512],
                                         skip_group_check=True)
                        sl = slice(bc * 512, (bc + 1) * 512)
                        nc.scalar.activation(h_out[:, sl], psm, AF.Relu,
                                             bias=bias2[:, 0:1])
                        nc.vector.bn_stats(bnst[:, bc, :], h_out_f[:, sl])
                    h_in, h_out = h_out, h_in

                # =========== head: out = h @ W_out.T + b_out ===========
                woutt_r = mp_.tile([h, N_OUT], f32r, bufs=1)
                nc.vector.tensor_copy(woutt_r, woutt_sb)
                out_row = mp_.tile([1, b_local], f32, bufs=1)
                for bc in range(BCN):
                    pso = mps.tile([1, 512], f32, tag="pso", name=f"pso{bc}")
                    nc.tensor.matmul(pso, woutt_r, h_in[:, bc * 512:(bc + 1) * 512],
                                     skip_group_check=True)
                    nc.scalar.activation(out_row[:, bc * 512:(bc + 1) * 512], pso,
                                         AF.Identity, bias=bout_sb[0:1, 0:1])
                nc.sync.dma_start(out_d.rearrange("b o -> o b"), out_row)
            bnp.release()
            hp.release()

    nc.compile()
    return nc


def make_in_maps(inputs, n_cores=N_CORES, b_local=B // N_CORES):
    """Host-side preprocessing: shard x, pre-transpose weights, replicate."""
    x = np.asarray(inputs["x"], np.float32)
    wint = np.ascontiguousarray(np.asarray(inputs["W_in"], np.float32).T)
    bin_ = np.asarray(inputs["b_in"], np.float32).reshape(-1, 1)
    grow = np.asarray(inputs["bn_gamma_in"], np.float32).reshape(1, -1)
    brow = np.asarray(inputs["bn_beta_in"], np.float32).reshape(1, -1)
    midwt = np.ascontiguousarray(
        np.asarray(inputs["mid_W"], np.float32).transpose(0, 2, 1))
    midg = np.ascontiguousarray(np.asarray(inputs["mid_gamma"], np.float32).T)
    midbeta = np.ascontiguousarray(np.asarray(inputs["mid_beta"], np.float32).T)
    midbias = np.ascontiguousarray(np.asarray(inputs["mid_b"], np.float32).T)
    woutt = np.ascontiguousarray(np.asarray(inputs["W_out"], np.float32).T)
    bout = np.asarray(inputs["b_out"], np.float32).reshape(1, 1)
    identf = np.eye(128, dtype=np.float32)
    identf2 = np.eye(2, dtype=np.float32)
    onesbf = np.ones((128, 1), dtype=BF16)

    common = dict(wint=wint, bin=bin_, grow=grow, brow=brow, midwt=midwt,
                  midg=midg, midbeta=midbeta, midbias=midbias, woutt=woutt,
                  bout=bout, identf=identf, identf2=identf2, onesbf=onesbf)
    in_maps = []
    for c in range(n_cores):
        m = dict(common)
        m["x"] = np.ascontiguousarray(x[c * b_local:(c + 1) * b_local])
        in_maps.append(m)
    return in_maps


@functools.lru_cache(maxsize=1)
def _get_program():
    return build_program()


def kernel(**inputs) -> np.ndarray:
    from concourse.bass_utils import run_bass_kernel_spmd
    nc = _get_program()
    in_maps = make_in_maps(inputs)
    res = run_bass_kernel_spmd(nc, in_maps, core_ids=list(range(N_CORES)))
    out = np.concatenate([res.results[c]["out"] for c in range(N_CORES)], axis=0)
    return out.astype(np.float32)


if __name__ == "__main__":
    nc = build_program(n_cores=2, b_local=1024, d=512, n_layers=2)
    print("built ok:", len(nc.inst_map), "instructions")



# revision 9
# speedup vs baseline: 1.2599x; 1.2599x over previous
"""Trainium2 Bass kernel for nn_Net_66408784331557 (dense MLP with sync-BN).

Reference computation:
    h = BN_train(x; gamma_in, beta_in)            # x: [65536, 2048]
    h = relu(h @ W_in.T + b_in)                   # -> [65536, 75]
    12x: h = relu(BN_train(h; g_l, b_l) @ W_l.T + bias_l)
    out = h @ W_out.T + b_out                     # -> [65536, 1]

Strategy (v2): data-parallel over the batch across 8 NeuronCores (8192 rows
each).  The host ships each core its batch shard already TRANSPOSED and cast
to bf16: xt [2048, 8192].  That kills all on-device PE transposes and
PSUM->SBUF copies of the baseline and halves HBM traffic (32MB/core/pass).

BatchNorm folds into the following Linear:
    BN(x) @ W.T = x @ (W*s).T + (beta - mu*s) @ W.T,  s = gamma*rsqrt(var+eps)
Pass 1 streams xt and computes per-feature (mean, E[x^2]) with DVE bn_stats
(feature-on-partition layout makes the batch the free axis).  One AllReduce
of the [128,16,2] stats, fold weights to bf16, then pass 2 re-streams xt and
does the 2048->75 matmul directly (bf16 stationary wfold, bf16 moving xt).

The 12 middle layers keep h resident in SBUF as [75, 8192] f32r; each layer:
matmul + fused bias/ReLU (scalar) + bn_stats (DVE), one tiny [75,2]
AllReduce, fold, next.  The head (75->1) is fused into layer 12's loop.
"""

import sys
import functools

import numpy as np

for _p in ("/opt/trn_rl_repo",):
    if _p not in sys.path:
        sys.path.insert(0, _p)

import ml_dtypes

N_CORES = 8
B = 65536
D = 2048
H = 75
L = 12
N_OUT = 1
EPS = 1e-5

F16 = np.float16

GW = 1024              # batch-group width (cols per stream tile)
SP_PAYLOAD = True      # single_packet on tiny sync-payload DMAs
PREWARM_AR = True      # dummy AllReduce early to absorb first-collective cost


def build_program(n_cores=N_CORES, b_local=B // N_CORES, d=D, h=H, n_layers=L,
                  debug=False):
    """Builds the SPMD Bass/Tile program (identical on every core)."""
    import concourse.bass as bass
    import concourse.mybir as mybir
    import concourse.tile as tile
    from concourse import bacc

    f32 = mybir.dt.float32
    f32r = mybir.dt.float32r
    bf16 = mybir.dt.bfloat16
    f16 = mybir.dt.float16
    AF = mybir.ActivationFunctionType
    ALU = mybir.AluOpType

    QD = d // 128          # feature blocks of 128 partitions
    NG = b_local // GW     # batch groups
    NCH = b_local // 512   # bn_stats chunks per feature block
    B_TOT = n_cores * b_local

    nc = bacc.Bacc("TRN2", target_bir_lowering=False, debug=debug,
                   enable_asserts=True, num_devices=n_cores)

    # ---- I/O ----
    xt_d = nc.dram_tensor("xt", [d, b_local], f16, kind="ExternalInput").ap()
    wint_d = nc.dram_tensor("wint", [d, h], f32, kind="ExternalInput").ap()
    bin_d = nc.dram_tensor("bin", [h, 1], f32, kind="ExternalInput").ap()
    growp_d = nc.dram_tensor("growp", [128, QD], f32, kind="ExternalInput").ap()
    browp_d = nc.dram_tensor("browp", [128, QD], f32, kind="ExternalInput").ap()
    midwt_d = nc.dram_tensor("midwt", [n_layers, h, h], f32, kind="ExternalInput").ap()
    midg_d = nc.dram_tensor("midg", [h, n_layers], f32, kind="ExternalInput").ap()
    midbeta_d = nc.dram_tensor("midbeta", [h, n_layers], f32, kind="ExternalInput").ap()
    midbias_d = nc.dram_tensor("midbias", [h, n_layers], f32, kind="ExternalInput").ap()
    woutt_d = nc.dram_tensor("woutt", [h, N_OUT], f32, kind="ExternalInput").ap()
    bout_d = nc.dram_tensor("bout", [1, 1], f32, kind="ExternalInput").ap()
    onesbf_d = nc.dram_tensor("onesbf", [128, 1], bf16, kind="ExternalInput").ap()
    out_d = nc.dram_tensor("out", [b_local, N_OUT], f32, kind="ExternalOutput").ap()

    rg = [list(range(n_cores))]

    with tile.TileContext(nc) as tc:
        with tc.tile_pool(name="const", bufs=1) as cp, \
             tc.tile_pool(name="drp", bufs=1, space="DRAM") as drp:

            # ---- constants into SBUF ----
            wint_sb = cp.tile([128, QD, h], f32)
            nc.sync.dma_start(wint_sb, wint_d.rearrange("(q p) h -> p q h", p=128))
            bin_sb = cp.tile([h, 1], f32)
            nc.sync.dma_start(bin_sb, bin_d)
            growp = cp.tile([128, QD], f32)
            nc.sync.dma_start(growp, growp_d)
            browp = cp.tile([128, QD], f32)
            nc.sync.dma_start(browp, browp_d)
            midwt_sb = cp.tile([h, n_layers, h], f32)
            nc.sync.dma_start(midwt_sb, midwt_d.rearrange("l k o -> k l o"))
            midg_sb = cp.tile([h, n_layers], f32)
            nc.sync.dma_start(midg_sb, midg_d)
            midbeta_sb = cp.tile([h, n_layers], f32)
            nc.sync.dma_start(midbeta_sb, midbeta_d)
            midbias_sb = cp.tile([h, n_layers], f32)
            nc.sync.dma_start(midbias_sb, midbias_d)
            woutt_sb = cp.tile([h, N_OUT], f32)
            nc.sync.dma_start(woutt_sb, woutt_d)
            bout_sb = cp.tile([1, 1], f32)
            nc.sync.dma_start(bout_sb, bout_d)
            onesbf = cp.tile([128, 1], bf16)
            nc.sync.dma_start(onesbf, onesbf_d)

            # prewarm the collective path: first collective on the NEFF pays
            # a large setup cost; pay it during pass 1 instead of at sync 1.
            if PREWARM_AR:
                wrm_i = drp.tile([1, 2], f32, name="wrm_i")
                wrm_o = drp.tile([1, 2], f32, name="wrm_o")
                nc.gpsimd.collective_compute(
                    "AllReduce", mybir.AluOpType.add, replica_groups=rg,
                    ins=[wrm_i.opt()], outs=[wrm_o.opt()])

            def dummy_warm(pool, name):
                # tiny bf16 matmul to keep the PE HAM clock-gate warm
                pd = pool.tile([1, 1], f32, tag="pdum", name=name)
                nc.tensor.matmul(pd, onesbf, onesbf, skip_group_check=True)

            # long-lived pools first (pool releases must be LIFO)
            hp = tc.alloc_tile_pool(name="hpool", bufs=1)
            h_a = hp.tile([h, b_local], f32r)
            h_b = hp.tile([h, b_local], f32r)
            bnp = tc.alloc_tile_pool(name="bnp", bufs=2)
            xp = tc.alloc_tile_pool(name="xp", bufs=2 * QD)
            # fold-phase scratch pool (released before pass 2)
            sp = tc.alloc_tile_pool(name="fold", bufs=1)

            # =========== PASS 1: per-feature mean / E[x^2] of x ===========
            bnst1 = sp.tile([128, QD, NCH, 6], f32)
            for g in range(NG):
                for q in range(QD):
                    t = xp.tile([128, GW], f16, tag="xt", name=f"p1_{g}_{q}")
                    nc.sync.dma_start(t, xt_d[q * 128:(q + 1) * 128,
                                              g * GW:(g + 1) * GW])
                    for c in range(GW // 512):
                        nc.vector.bn_stats(bnst1[:, q, g * (GW // 512) + c, :],
                                           t[:, c * 512:(c + 1) * 512])

            mvq = sp.tile([128, QD, 2], f32)
            for q in range(QD):
                nc.vector.bn_aggr(mvq[:, q, :], bnst1[:, q, :, :])
            # payload = (mean, E[x^2]) so the AllReduce's sum/n_cores is exact
            pay = sp.tile([128, QD, 2], f32)
            msq = sp.tile([128, QD], f32)
            nc.vector.tensor_copy(pay[:, :, 0], mvq[:, :, 0])
            nc.vector.tensor_tensor(out=msq, in0=mvq[:, :, 0], in1=mvq[:, :, 0],
                                    op=ALU.mult)
            nc.vector.tensor_tensor(out=pay[:, :, 1], in0=mvq[:, :, 1], in1=msq,
                                    op=ALU.add)

            st1i = drp.tile([128, QD, 2], f32)
            st1o = drp.tile([128, QD, 2], f32)
            nc.scalar.dma_start(st1i, pay)
            nc.gpsimd.collective_compute(
                "AllReduce", mybir.AluOpType.add, replica_groups=rg,
                ins=[st1i.opt()], outs=[st1o.opt()])
            g_row = pay
            nc.scalar.dma_start(g_row, st1o)

            # ---- stats -> (s, t) in [128, QD] layout; Newton-polished rsqrt
            mu = sp.tile([128, QD], f32)
            tmp1 = sp.tile([128, QD], f32)
            tmp2 = sp.tile([128, QD], f32)
            tmp3 = sp.tile([128, QD], f32)
            vep = sp.tile([128, QD], f32)
            nc.vector.tensor_scalar_mul(mu, g_row[:, :, 0], 1.0 / n_cores)
            nc.vector.tensor_scalar(out=vep, in0=g_row[:, :, 1],
                                    scalar1=1.0 / n_cores, scalar2=float(EPS),
                                    op0=ALU.mult, op1=ALU.add)  # E[x^2]+eps
            nc.vector.tensor_tensor(out=tmp1, in0=mu, in1=mu, op=ALU.mult)
            nc.vector.tensor_tensor(out=vep, in0=vep, in1=tmp1,
                                    op=ALU.subtract)  # var+eps
            nc.scalar.activation(tmp2, vep, AF.Sqrt)
            nc.vector.reciprocal(tmp3, tmp2)
            # one Newton step: r = r0*(1.5 - 0.5*vep*r0^2)
            nc.vector.tensor_tensor(out=tmp1, in0=tmp3, in1=tmp3, op=ALU.mult)
            nc.vector.tensor_tensor(out=tmp2, in0=vep, in1=tmp1, op=ALU.mult)
            nc.vector.tensor_scalar(out=tmp2, in0=tmp2, scalar1=-0.5, scalar2=1.5,
                                    op0=ALU.mult, op1=ALU.add)
            nc.vector.tensor_tensor(out=tmp1, in0=tmp3, in1=tmp2, op=ALU.mult)  # r
            s_p = tmp3
            nc.vector.tensor_tensor(out=s_p, in0=tmp1, in1=growp, op=ALU.mult)
            nc.vector.tensor_tensor(out=tmp2, in0=mu, in1=s_p, op=ALU.mult)
            t_p = tmp1
            nc.vector.tensor_tensor(out=t_p, in0=browp, in1=tmp2,
                                    op=ALU.subtract)  # t = beta - mu*s

            # fold: wfold[:,q,:] = bf16(wint[:,q,:] * s[:,q]); bias1 = b_in + W@t
            wfold = cp.tile([128, QD, h], f16)
            for q in range(QD):
                nc.vector.tensor_scalar_mul(wfold[:, q, :], wint_sb[:, q, :],
                                            s_p[:, q:q + 1])
            with tc.tile_pool(name="pbias", bufs=1, space="PSUM") as pbias:
                ps_b1 = pbias.tile([h, 1], f32)
                for q in range(QD):
                    nc.tensor.matmul(ps_b1, wint_sb[:, q, :], t_p[:, q:q + 1],
                                     start=(q == 0), stop=(q == QD - 1),
                                     skip_group_check=True)
                bias1 = cp.tile([h, 1], f32)
                nc.vector.tensor_tensor(out=bias1, in0=ps_b1, in1=bin_sb, op=ALU.add)
            sp.release()

            bnst = bnp.tile([h, NCH, 6], f32, tag="bnst", name="bnst_l0")

            # =========== PASS 2: h1 = relu(xn @ W_in'.T + bias1) ===========
            h_a_f = h_a.bitcast(f32)
            with tc.tile_pool(name="p2ps", bufs=3, space="PSUM") as p2ps, \
                 tc.tile_pool(name="p2pd", bufs=1, space="PSUM") as p2pd:
                for g in range(NG):
                    tiles = []
                    for q in range(QD):
                        t = xp.tile([128, GW], f16, tag="xt", name=f"p2_{g}_{q}")
                        nc.sync.dma_start(t, xt_d[q * 128:(q + 1) * 128,
                                                  g * GW:(g + 1) * GW])
                        tiles.append(t)
                    psh = p2ps.tile([h, GW], f32, tag="psh", name=f"psh{g}")
                    for q in range(QD):
                        nc.tensor.matmul(psh[:, 0:512], wfold[:, q, :],
                                         tiles[q][:, 0:512],
                                         start=(q == 0), stop=(q == QD - 1),
                                         skip_group_check=True)
                        nc.tensor.matmul(psh[:, 512:GW], wfold[:, q, :],
                                         tiles[q][:, 512:GW],
                                         start=(q == 0), stop=(q == QD - 1),
                                         skip_group_check=True)
                    sl = slice(g * GW, (g + 1) * GW)
                    nc.scalar.activation(h_a[:, sl], psh, AF.Relu,
                                         bias=bias1[:, 0:1])
                    for c in range(GW // 512):
                        cc = g * (GW // 512) + c
                        nc.vector.bn_stats(bnst[:, cc, :],
                                           h_a_f[:, cc * 512:(cc + 1) * 512])
                    dummy_warm(p2pd, f"dum2_{g}")
            xp.release()

            # =========== 12 middle layers (+ head fused into the last) =====
            h_in, h_out = h_a, h_b
            with tc.tile_pool(name="mid", bufs=2) as mp_, \
                 tc.tile_pool(name="midps", bufs=2, space="PSUM") as mps, \
                 tc.tile_pool(name="midpso", bufs=1, space="PSUM") as mpso, \
                 tc.tile_pool(name="midpb", bufs=1, space="PSUM") as mpb, \
                 tc.tile_pool(name="midpd", bufs=1, space="PSUM") as mpd:
                woutt_r = mp_.tile([h, N_OUT], f32r, bufs=1)
                nc.vector.tensor_copy(woutt_r, woutt_sb)
                out_row = mp_.tile([1, b_local], f32, bufs=1)

                for l in range(n_layers):
                    mv = mp_.tile([h, 2], f32, tag="mv", name=f"mv{l}")
                    nc.vector.bn_aggr(mv, bnst)
                    # payload [mean, E[h^2]]
                    pay2 = mp_.tile([h, 2], f32, tag="pay", name=f"pay{l}")
                    nc.vector.tensor_copy(pay2[:, 0:1], mv[:, 0:1])
                    msq2 = mp_.tile([h, 1], f32, tag="msq", name=f"msq{l}")
                    nc.vector.tensor_tensor(out=msq2, in0=mv[:, 0:1], in1=mv[:, 0:1],
                                            op=ALU.mult)
                    nc.vector.tensor_tensor(out=pay2[:, 1:2], in0=mv[:, 1:2],
                                            in1=msq2, op=ALU.add)
                    mbi = drp.tile([h, 2], f32, name=f"mbi{l}")
                    mbo = drp.tile([h, 2], f32, name=f"mbo{l}")
                    nc.scalar.dma_start(mbi, pay2, single_packet=SP_PAYLOAD)
                    nc.gpsimd.collective_compute(
                        "AllReduce", mybir.AluOpType.add, replica_groups=rg,
                        ins=[mbi.opt()], outs=[mbo.opt()])
                    g2 = mp_.tile([h, 2], f32, tag="g2", name=f"g2{l}")
                    nc.scalar.dma_start(g2, mbo, single_packet=SP_PAYLOAD)

                    # global mean/var -> s2, t2
                    dg = mp_.tile([h, 2], f32, tag="dg", name=f"dg{l}")
                    nc.vector.tensor_scalar_mul(dg, g2, 1.0 / n_cores)
                    musq2 = mp_.tile([h, 1], f32, tag="musq2", name=f"musq2{l}")
                    nc.vector.tensor_tensor(out=musq2, in0=dg[:, 0:1],
                                            in1=dg[:, 0:1], op=ALU.mult)
                    vef = mp_.tile([h, 1], f32, tag="vef", name=f"vef{l}")
                    # vef = (E[h^2] + eps) - mean^2
                    nc.vector.scalar_tensor_tensor(
                        out=vef, in0=dg[:, 1:2], scalar=float(EPS), in1=musq2,
                        op0=ALU.add, op1=ALU.subtract)
                    sd2 = mp_.tile([h, 1], f32, tag="sd2", name=f"sd2{l}")
                    nc.scalar.activation(sd2, vef, AF.Sqrt)
                    rr = mp_.tile([h, 1], f32, tag="rr", name=f"rr{l}")
                    nc.vector.reciprocal(rr, sd2)
                    s2 = mp_.tile([h, 1], f32, tag="s2", name=f"s2{l}")
                    nc.vector.tensor_tensor(out=s2, in0=rr, in1=midg_sb[:, l:l + 1],
                                            op=ALU.mult)
                    mt = mp_.tile([h, 1], f32, tag="mt", name=f"mt{l}")
                    nc.vector.tensor_tensor(out=mt, in0=dg[:, 0:1], in1=s2,
                                            op=ALU.mult)
                    t2 = mp_.tile([h, 1], f32, tag="t2", name=f"t2{l}")
                    nc.vector.tensor_tensor(out=t2, in0=midbeta_sb[:, l:l + 1],
                                            in1=mt, op=ALU.subtract)
                    wf = mp_.tile([h, h], f32r, tag="wf", name=f"wf{l}")
                    nc.vector.tensor_scalar_mul(wf, midwt_sb[:, l, :], s2)
                    ps_b2 = mpb.tile([h, 1], f32, tag="psb2", name=f"psb2_{l}")
                    nc.tensor.matmul(ps_b2, midwt_sb[:, l, :], t2,
                                     skip_group_check=True)
                    bias2 = mp_.tile([h, 1], f32, tag="bias2", name=f"bias2{l}")
                    nc.vector.tensor_tensor(out=bias2, in0=ps_b2,
                                            in1=midbias_sb[:, l:l + 1], op=ALU.add)

                    last = (l == n_layers - 1)
                    if not last:
                        bnst = bnp.tile([h, NCH, 6], f32, tag="bnst",
                                        name=f"bnst_l{l + 1}")
                    h_out_f = h_out.bitcast(f32)
                    for g in range(NG):
                        psm = mps.tile([h, GW], f32, tag="psm", name=f"psm{l}_{g}")
                        if g % 4 == 0:
                            dummy_warm(mpd, f"mdum{l}_{g}")
                        nc.tensor.matmul(psm[:, 0:512], wf,
                                         h_in[:, g * GW:g * GW + 512],
                                         skip_group_check=True)
                        nc.tensor.matmul(psm[:, 512:GW], wf,
                                         h_in[:, g * GW + 512:(g + 1) * GW],
                                         skip_group_check=True)
                        sl = slice(g * GW, (g + 1) * GW)
                        nc.scalar.activation(h_out[:, sl], psm, AF.Relu,
                                             bias=bias2[:, 0:1])
                        if not last:
                            for c in range(GW // 512):
                                cc = g * (GW // 512) + c
                                nc.vector.bn_stats(bnst[:, cc, :],
                                                   h_out_f[:, cc * 512:(cc + 1) * 512])
                        else:
                            # head fused into layer 12's group loop
                            pso = mpso.tile([1, GW], f32, tag="pso",
                                            name=f"pso{g}")
                            nc.tensor.matmul(pso[:, 0:512], woutt_r,
                                             h_out[:, g * GW:g * GW + 512],
                                             skip_group_check=True)
                            nc.tensor.matmul(pso[:, 512:GW], woutt_r,
                                             h_out[:, g * GW + 512:(g + 1) * GW],
                                             skip_group_check=True)
                            nc.scalar.activation(out_row[:, sl], pso, AF.Identity,
                                                 bias=bout_sb[0:1, 0:1])
                    h_in, h_out = h_out, h_in

                nc.sync.dma_start(out_d.rearrange("b o -> o b"), out_row)
            bnp.release()
            hp.release()

    nc.compile()
    return nc


def make_in_maps(inputs, n_cores=N_CORES, b_local=B // N_CORES):
    """Host-side layout prep: shard+transpose x to bf16, reshape tiny weights."""
    x = np.asarray(inputs["x"], np.float32)
    QD = D // 128
    xbf = x.astype(F16)  # one big cast, then per-core transpose
    wint = np.ascontiguousarray(np.asarray(inputs["W_in"], np.float32).T)
    bin_ = np.asarray(inputs["b_in"], np.float32).reshape(-1, 1)
    growp = np.ascontiguousarray(
        np.asarray(inputs["bn_gamma_in"], np.float32).reshape(QD, 128).T)
    browp = np.ascontiguousarray(
        np.asarray(inputs["bn_beta_in"], np.float32).reshape(QD, 128).T)
    midwt = np.ascontiguousarray(
        np.asarray(inputs["mid_W"], np.float32).transpose(0, 2, 1))
    midg = np.ascontiguousarray(np.asarray(inputs["mid_gamma"], np.float32).T)
    midbeta = np.ascontiguousarray(np.asarray(inputs["mid_beta"], np.float32).T)
    midbias = np.ascontiguousarray(np.asarray(inputs["mid_b"], np.float32).T)
    woutt = np.ascontiguousarray(np.asarray(inputs["W_out"], np.float32).T)
    bout = np.asarray(inputs["b_out"], np.float32).reshape(1, 1)
    onesbf = np.ones((128, 1), dtype=ml_dtypes.bfloat16)

    common = dict(wint=wint, bin=bin_, growp=growp, browp=browp, midwt=midwt,
                  midg=midg, midbeta=midbeta, midbias=midbias, woutt=woutt,
                  bout=bout, onesbf=onesbf)
    in_maps = []
    for c in range(n_cores):
        m = dict(common)
        m["xt"] = np.ascontiguousarray(xbf[c * b_local:(c + 1) * b_local].T)
        in_maps.append(m)
    return in_maps


@functools.lru_cache(maxsize=1)
def _get_program():
    return build_program()


def kernel(**inputs) -> np.ndarray:
    from concourse.bass_utils import run_bass_kernel_spmd
    nc = _get_program()
    in_maps = make_in_maps(inputs)
    res = run_bass_kernel_spmd(nc, in_maps, core_ids=list(range(N_CORES)))
    out = np.concatenate([res.results[c]["out"] for c in range(N_CORES)], axis=0)
    return out.astype(np.float32)


if __name__ == "__main__":
    nc = build_program(n_cores=2, b_local=2048, d=512, n_layers=2)
    print("built ok:", len(nc.inst_map), "instructions")


# revision 12
# speedup vs baseline: 1.4348x; 1.1388x over previous
"""Trainium2 Bass kernel for nn_Net_66408784331557 (dense MLP with sync-BN).

Reference computation:
    h = BN_train(x; gamma_in, beta_in)            # x: [65536, 2048]
    h = relu(h @ W_in.T + b_in)                   # -> [65536, 75]
    12x: h = relu(BN_train(h; g_l, b_l) @ W_l.T + bias_l)
    out = h @ W_out.T + b_out                     # -> [65536, 1]

Strategy (v3): data-parallel over the batch across 8 NeuronCores (8192 rows
each).  The host ships each core its shard twice in fp16: xb [8192, 2048]
(row-major) and xt [2048, 8192] (transposed).  All FLOPs stay on device.

Pass 1 streams xb and computes per-feature (sum, sumsq) on the PE with
ones-vector matmuls (squares from DVE, fp16 2x mode); one AllReduce of the
[128,16,2] sums; BN folds into the input Linear:
    BN(x) @ W.T = x @ (W*s).T + (beta - mu*s) @ W.T,  s = gamma*rsqrt(var+eps)
Pass 2 streams xt and runs the 2048->75 matmul directly (fp16 stationary
wfold, fp16 moving xt) -- no on-device transposes at all.  Pass-2 bulk DMA
for groups >= 2 is queued on the sync engine BEHIND a barrier DMA that reads
the AllReduce result, so the SDMA engines are quiet during the collective.

The 12 middle layers keep h in SBUF as [75, 8192] f32r; per layer: matmul +
fused bias/ReLU (scalar) + bn_stats (DVE), one tiny [75,2] AllReduce, fold.
The head (75->1) is fused into layer 12's loop.
"""

import sys
import functools

import numpy as np

for _p in ("/opt/trn_rl_repo",):
    if _p not in sys.path:
        sys.path.insert(0, _p)

import ml_dtypes

N_CORES = 8
B = 65536
D = 2048
H = 75
L = 12
N_OUT = 1
EPS = 1e-5

F16 = np.float16

GW = 1024              # p2 batch-group width (cols per stream tile)
PW = 2048              # p1 row-tile width (feature cols per tile)
SWDGE_PAYLOAD = True   # mid-layer sync payload DMA via gpsimd SWDGE queue
PREWARM_AR = True      # dummy AllReduce early to absorb first-collective cost
WARM_LINKS = 6         # DVE-chained PE warm anchors across the AR1 window


def build_program(n_cores=N_CORES, b_local=B // N_CORES, d=D, h=H, n_layers=L,
                  debug=False):
    """Builds the SPMD Bass/Tile program (identical on every core)."""
    import concourse.bass as bass
    import concourse.mybir as mybir
    import concourse.tile as tile
    from concourse import bacc

    f32 = mybir.dt.float32
    f32r = mybir.dt.float32r
    f16 = mybir.dt.float16
    AF = mybir.ActivationFunctionType
    ALU = mybir.AluOpType

    QD = d // 128          # feature blocks of 128 partitions
    NG = b_local // GW     # p2 batch groups
    NCH = b_local // 512   # bn_stats chunks per layer
    NBT = b_local // 128   # p1 row tiles (128 rows each)
    CC = d // 512          # p1 colsum chunks of 512
    B_TOT = n_cores * b_local

    nc = bacc.Bacc("TRN2", target_bir_lowering=False, debug=debug,
                   enable_asserts=True, num_devices=n_cores)

    # ---- I/O ----
    xb_d = nc.dram_tensor("xb", [b_local, d], f16, kind="ExternalInput").ap()
    xt_d = nc.dram_tensor("xt", [d, b_local], f16, kind="ExternalInput").ap()
    wint_d = nc.dram_tensor("wint", [d, h], f32, kind="ExternalInput").ap()
    bin_d = nc.dram_tensor("bin", [h, 1], f32, kind="ExternalInput").ap()
    growp_d = nc.dram_tensor("growp", [128, QD], f32, kind="ExternalInput").ap()
    browp_d = nc.dram_tensor("browp", [128, QD], f32, kind="ExternalInput").ap()
    midwt_d = nc.dram_tensor("midwt", [n_layers, h, h], f32, kind="ExternalInput").ap()
    midg_d = nc.dram_tensor("midg", [h, n_layers], f32, kind="ExternalInput").ap()
    midbeta_d = nc.dram_tensor("midbeta", [h, n_layers], f32, kind="ExternalInput").ap()
    midbias_d = nc.dram_tensor("midbias", [h, n_layers], f32, kind="ExternalInput").ap()
    woutt_d = nc.dram_tensor("woutt", [h, N_OUT], f32, kind="ExternalInput").ap()
    bout_d = nc.dram_tensor("bout", [1, 1], f32, kind="ExternalInput").ap()
    onesf_d = nc.dram_tensor("onesf", [128, 1], f16, kind="ExternalInput").ap()
    identf2_d = nc.dram_tensor("identf2", [2, 2], f32, kind="ExternalInput").ap()
    out_d = nc.dram_tensor("out", [b_local, N_OUT], f32, kind="ExternalOutput").ap()

    rg = [list(range(n_cores))]

    with tile.TileContext(nc) as tc:
        with tc.tile_pool(name="const", bufs=1) as cp, \
             tc.tile_pool(name="drp", bufs=1, space="DRAM") as drp:

            # ---- constants into SBUF ----
            wint_sb = cp.tile([128, QD, h], f32)
            nc.sync.dma_start(wint_sb, wint_d.rearrange("(q p) h -> p q h", p=128))
            bin_sb = cp.tile([h, 1], f32)
            nc.sync.dma_start(bin_sb, bin_d)
            growp = cp.tile([128, QD], f32)
            nc.sync.dma_start(growp, growp_d)
            browp = cp.tile([128, QD], f32)
            nc.sync.dma_start(browp, browp_d)
            midwt_sb = cp.tile([h, n_layers, h], f32)
            nc.sync.dma_start(midwt_sb, midwt_d.rearrange("l k o -> k l o"))
            midg_sb = cp.tile([h, n_layers], f32)
            nc.sync.dma_start(midg_sb, midg_d)
            midbeta_sb = cp.tile([h, n_layers], f32)
            nc.sync.dma_start(midbeta_sb, midbeta_d)
            midbias_sb = cp.tile([h, n_layers], f32)
            nc.sync.dma_start(midbias_sb, midbias_d)
            woutt_sb = cp.tile([h, N_OUT], f32)
            nc.sync.dma_start(woutt_sb, woutt_d)
            bout_sb = cp.tile([1, 1], f32)
            nc.sync.dma_start(bout_sb, bout_d)
            onesf = cp.tile([128, 1], f16)
            nc.sync.dma_start(onesf, onesf_d)
            identf2 = cp.tile([2, 2], f32)
            nc.sync.dma_start(identf2, identf2_d)
            warmj = cp.tile([128, 2 * PW], f16)  # warm-bridge ping-pong buffer

            # prewarm the collective path during pass 1
            if PREWARM_AR:
                wrm_i = drp.tile([1, 2], f32, name="wrm_i")
                wrm_o = drp.tile([1, 2], f32, name="wrm_o")
                nc.gpsimd.collective_compute(
                    "AllReduce", mybir.AluOpType.add, replica_groups=rg,
                    ins=[wrm_i.opt()], outs=[wrm_o.opt()])

            def dummy_warm(pool, name, dep=None):
                # tiny fp16 matmul keeps the PE HAM clock-gate warm; with
                # dep, it fires only after that tile is written
                pd = pool.tile([1, 1], f32, tag="pdum", name=name)
                src = dep if dep is not None else onesf
                nc.tensor.matmul(pd, src[0:128, 0:1], src[0:128, 0:1],
                                 skip_group_check=True)

            # long-lived pools first (pool releases must be LIFO)
            hp = tc.alloc_tile_pool(name="hpool", bufs=1)
            h_a = hp.tile([h, b_local], f32r)
            h_b = hp.tile([h, b_local], f32r)
            bnp = tc.alloc_tile_pool(name="bnp", bufs=2)
            xp = tc.alloc_tile_pool(name="xp", bufs=2 * QD)

            # =========== PASS 2 PREFETCH (groups 0-1, runs during pass 1) ===
            p2_tiles = {}
            for g in range(2):
                for q in range(QD):
                    t = xp.tile([128, GW], f16, tag="xt", name=f"p2_{g}_{q}")
                    nc.sync.dma_start(t, xt_d[q * 128:(q + 1) * 128,
                                              g * GW:(g + 1) * GW])
                    p2_tiles[(g, q)] = t

            # =========== PASS 1: per-feature sum / sumsq of x (PE) =========
            xbp = tc.alloc_tile_pool(name="xbp", bufs=4)
            xsp = tc.alloc_tile_pool(name="xsp", bufs=3)
            sp = tc.alloc_tile_pool(name="fold", bufs=1)
            with tc.tile_pool(name="p1ps", bufs=1, space="PSUM") as p1ps:
                ps_sum = p1ps.tile([1, d], f32)
                ps_sq = p1ps.tile([1, d], f32)
                for i in range(NBT):
                    xf = xbp.tile([128, d], f16, tag="xb", name=f"xb{i}")
                    nc.sync.dma_start(xf, xb_d[i * 128:(i + 1) * 128, :])
                    xsq = xsp.tile([128, d], f16, tag="xsq", name=f"xsq{i}")
                    nc.vector.tensor_tensor(out=xsq, in0=xf, in1=xf, op=ALU.mult)
                    for c in range(CC):
                        nc.tensor.matmul(ps_sum[:, c * 512:(c + 1) * 512], onesf,
                                         xf[:, c * 512:(c + 1) * 512],
                                         start=(i == 0), stop=(i == NBT - 1),
                                         skip_group_check=True)
                    for c in range(CC):
                        nc.tensor.matmul(ps_sq[:, c * 512:(c + 1) * 512], onesf,
                                         xsq[:, c * 512:(c + 1) * 512],
                                         start=(i == 0), stop=(i == NBT - 1),
                                         skip_group_check=True)

                # psum rows -> SBUF rows (scalar + vector in parallel)
                sum_row = sp.tile([1, d], f32)
                sq_row = sp.tile([1, d], f32)
                nc.scalar.copy(sum_row, ps_sum)
                nc.vector.tensor_copy(sq_row, ps_sq)

            # transpose rows into per-partition layout [128, QD, 2]
            pay = sp.tile([128, QD, 2], f32)
            with tc.tile_pool(name="stps", bufs=1, space="PSUM") as stps:
                ps_st = stps.tile([128, QD, 2], f32)
                for q in range(QD):
                    nc.tensor.matmul(ps_st[:, q, 0:1],
                                     sum_row[:, q * 128:(q + 1) * 128],
                                     identf2[0:1, 0:1], is_transpose=True,
                                     skip_group_check=True)
                    nc.tensor.matmul(ps_st[:, q, 1:2],
                                     sq_row[:, q * 128:(q + 1) * 128],
                                     identf2[0:1, 0:1], is_transpose=True,
                                     skip_group_check=True)
                nc.vector.tensor_copy(pay, ps_st)

            st1i = drp.tile([128, QD, 2], f32)
            st1o = drp.tile([128, QD, 2], f32)
            nc.scalar.dma_start(st1i, pay)
            nc.gpsimd.collective_compute(
                "AllReduce", mybir.AluOpType.add, replica_groups=rg,
                ins=[st1i.opt()], outs=[st1o.opt()])
            g_row = pay
            nc.scalar.dma_start(g_row, st1o)

            # PE warm bridge across the collective window: DVE copy chain
            # anchored on the payload, each link feeding a tiny PE matmul.
            with tc.tile_pool(name="wmps", bufs=1, space="PSUM") as wmps:
                nc.vector.tensor_copy(warmj[:, 0:2], pay[:, 0, 0:1].bitcast(f16))
                for wl in range(WARM_LINKS):
                    a = (wl % 2) * PW
                    b2 = ((wl + 1) % 2) * PW
                    dummy_warm(wmps, f"wm{wl}", dep=warmj[:, a:a + 1])
                    nc.vector.tensor_copy(warmj[:, b2:b2 + PW], warmj[:, a:a + PW])

            # ---- global stats -> (s, t) in [128, QD]; Newton-polished rsqrt
            mu = sp.tile([128, QD], f32)
            tmp1 = sp.tile([128, QD], f32)
            tmp2 = sp.tile([128, QD], f32)
            tmp3 = sp.tile([128, QD], f32)
            vep = sp.tile([128, QD], f32)
            nc.vector.tensor_scalar_mul(mu, g_row[:, :, 0], 1.0 / B_TOT)
            nc.vector.tensor_scalar(out=vep, in0=g_row[:, :, 1],
                                    scalar1=1.0 / B_TOT, scalar2=float(EPS),
                                    op0=ALU.mult, op1=ALU.add)  # E[x^2]+eps
            nc.vector.tensor_tensor(out=tmp1, in0=mu, in1=mu, op=ALU.mult)
            nc.vector.tensor_tensor(out=vep, in0=vep, in1=tmp1,
                                    op=ALU.subtract)  # var+eps
            nc.scalar.activation(tmp2, vep, AF.Sqrt)
            nc.vector.reciprocal(tmp3, tmp2)
            nc.vector.tensor_tensor(out=tmp1, in0=tmp3, in1=tmp3, op=ALU.mult)
            nc.vector.tensor_tensor(out=tmp2, in0=vep, in1=tmp1, op=ALU.mult)
            nc.vector.tensor_scalar(out=tmp2, in0=tmp2, scalar1=-0.5, scalar2=1.5,
                                    op0=ALU.mult, op1=ALU.add)
            nc.vector.tensor_tensor(out=tmp1, in0=tmp3, in1=tmp2, op=ALU.mult)  # r
            s_p = tmp3
            nc.vector.tensor_tensor(out=s_p, in0=tmp1, in1=growp, op=ALU.mult)
            nc.vector.tensor_tensor(out=tmp2, in0=mu, in1=s_p, op=ALU.mult)
            t_p = tmp1
            nc.vector.tensor_tensor(out=t_p, in0=browp, in1=tmp2,
                                    op=ALU.subtract)  # t = beta - mu*s

            # fold: wfold = f16(wint * s); bias1 = b_in + W_in @ t
            wfold = cp.tile([128, QD, h], f16)
            for q in range(QD):
                nc.vector.tensor_scalar_mul(wfold[:, q, :], wint_sb[:, q, :],
                                            s_p[:, q:q + 1])
            with tc.tile_pool(name="pbias", bufs=1, space="PSUM") as pbias:
                ps_b1 = pbias.tile([h, 1], f32)
                for q in range(QD):
                    nc.tensor.matmul(ps_b1, wint_sb[:, q, :], t_p[:, q:q + 1],
                                     start=(q == 0), stop=(q == QD - 1),
                                     skip_group_check=True)
                bias1 = cp.tile([h, 1], f32)
                nc.vector.tensor_tensor(out=bias1, in0=ps_b1, in1=bin_sb, op=ALU.add)
            sp.release()
            xsp.release()
            xbp.release()

            # barrier: sync-queue DMA that reads the AllReduce result; all
            # later sync-queue DMAs (p2 groups 2+) queue FIFO behind it, so
            # the SDMA engines are quiet while the collective runs.
            bar_d = drp.tile([1, 2], f32, name="bar_d")
            nc.sync.dma_start(bar_d, g_row[0:1, 0, :])

            bnst = bnp.tile([h, NCH, 6], f32, tag="bnst", name="bnst_l0")

            # =========== PASS 2: h1 = relu(xn @ W_in'.T + bias1) ===========
            h_a_f = h_a.bitcast(f32)
            with tc.tile_pool(name="p2ps", bufs=3, space="PSUM") as p2ps, \
                 tc.tile_pool(name="p2pd", bufs=1, space="PSUM") as p2pd:
                for g in range(NG):
                    tiles = []
                    for q in range(QD):
                        if (g, q) in p2_tiles:
                            tiles.append(p2_tiles[(g, q)])
                            continue
                        t = xp.tile([128, GW], f16, tag="xt", name=f"p2_{g}_{q}")
                        nc.sync.dma_start(t, xt_d[q * 128:(q + 1) * 128,
                                                  g * GW:(g + 1) * GW])
                        tiles.append(t)
                    psh = p2ps.tile([h, GW], f32, tag="psh", name=f"psh{g}")
                    for q in range(QD):
                        nc.tensor.matmul(psh[:, 0:512], wfold[:, q, :],
                                         tiles[q][:, 0:512],
                                         start=(q == 0), stop=(q == QD - 1),
                                         skip_group_check=True)
                        nc.tensor.matmul(psh[:, 512:GW], wfold[:, q, :],
                                         tiles[q][:, 512:GW],
                                         start=(q == 0), stop=(q == QD - 1),
                                         skip_group_check=True)
                    sl = slice(g * GW, (g + 1) * GW)
                    nc.scalar.activation(h_a[:, sl], psh, AF.Relu,
                                         bias=bias1[:, 0:1])
                    for c in range(GW // 512):
                        cc = g * (GW // 512) + c
                        nc.vector.bn_stats(bnst[:, cc, :],
                                           h_a_f[:, cc * 512:(cc + 1) * 512])
                    dummy_warm(p2pd, f"dum2_{g}")
            xp.release()

            # =========== 12 middle layers (+ head fused into the last) =====
            h_in, h_out = h_a, h_b
            with tc.tile_pool(name="mid", bufs=2) as mp_, \
                 tc.tile_pool(name="midps", bufs=2, space="PSUM") as mps, \
                 tc.tile_pool(name="midpso", bufs=1, space="PSUM") as mpso, \
                 tc.tile_pool(name="midpb", bufs=1, space="PSUM") as mpb, \
                 tc.tile_pool(name="midpd", bufs=1, space="PSUM") as mpd:
                woutt_r = mp_.tile([h, N_OUT], f32r, bufs=1)
                nc.vector.tensor_copy(woutt_r, woutt_sb)
                out_row = mp_.tile([1, b_local], f32, bufs=1)

                for l in range(n_layers):
                    mv = mp_.tile([h, 2], f32, tag="mv", name=f"mv{l}")
                    nc.vector.bn_aggr(mv, bnst)
                    pay2 = mp_.tile([h, 2], f32, tag="pay", name=f"pay{l}")
                    nc.vector.tensor_copy(pay2[:, 0:1], mv[:, 0:1])
                    msq2 = mp_.tile([h, 1], f32, tag="msq", name=f"msq{l}")
                    nc.vector.tensor_tensor(out=msq2, in0=mv[:, 0:1], in1=mv[:, 0:1],
                                            op=ALU.mult)
                    nc.vector.tensor_tensor(out=pay2[:, 1:2], in0=mv[:, 1:2],
                                            in1=msq2, op=ALU.add)
                    mbi = drp.tile([h, 2], f32, name=f"mbi{l}")
                    mbo = drp.tile([h, 2], f32, name=f"mbo{l}")
                    if SWDGE_PAYLOAD:
                        nc.gpsimd.dma_start(mbi, pay2)
                    else:
                        nc.scalar.dma_start(mbi, pay2)
                    nc.gpsimd.collective_compute(
                        "AllReduce", mybir.AluOpType.add, replica_groups=rg,
                        ins=[mbi.opt()], outs=[mbo.opt()])
                    g2 = mp_.tile([h, 2], f32, tag="g2", name=f"g2{l}")
                    nc.scalar.dma_start(g2, mbo)

                    dg = mp_.tile([h, 2], f32, tag="dg", name=f"dg{l}")
                    nc.vector.tensor_scalar_mul(dg, g2, 1.0 / n_cores)
                    musq2 = mp_.tile([h, 1], f32, tag="musq2", name=f"musq2{l}")
                    nc.vector.tensor_tensor(out=musq2, in0=dg[:, 0:1],
                                            in1=dg[:, 0:1], op=ALU.mult)
                    vef = mp_.tile([h, 1], f32, tag="vef", name=f"vef{l}")
                    nc.vector.scalar_tensor_tensor(
                        out=vef, in0=dg[:, 1:2], scalar=float(EPS), in1=musq2,
                        op0=ALU.add, op1=ALU.subtract)
                    sd2 = mp_.tile([h, 1], f32, tag="sd2", name=f"sd2{l}")
                    nc.scalar.activation(sd2, vef, AF.Sqrt)
                    rr = mp_.tile([h, 1], f32, tag="rr", name=f"rr{l}")
                    nc.vector.reciprocal(rr, sd2)
                    s2 = mp_.tile([h, 1], f32, tag="s2", name=f"s2{l}")
                    nc.vector.tensor_tensor(out=s2, in0=rr, in1=midg_sb[:, l:l + 1],
                                            op=ALU.mult)
                    mt = mp_.tile([h, 1], f32, tag="mt", name=f"mt{l}")
                    nc.vector.tensor_tensor(out=mt, in0=dg[:, 0:1], in1=s2,
                                            op=ALU.mult)
                    t2 = mp_.tile([h, 1], f32, tag="t2", name=f"t2{l}")
                    nc.vector.tensor_tensor(out=t2, in0=midbeta_sb[:, l:l + 1],
                                            in1=mt, op=ALU.subtract)
                    wf = mp_.tile([h, h], f32r, tag="wf", name=f"wf{l}")
                    nc.vector.tensor_scalar_mul(wf, midwt_sb[:, l, :], s2)
                    ps_b2 = mpb.tile([h, 1], f32, tag="psb2", name=f"psb2_{l}")
                    nc.tensor.matmul(ps_b2, midwt_sb[:, l, :], t2,
                                     skip_group_check=True)
                    bias2 = mp_.tile([h, 1], f32, tag="bias2", name=f"bias2{l}")
                    nc.vector.tensor_tensor(out=bias2, in0=ps_b2,
                                            in1=midbias_sb[:, l:l + 1], op=ALU.add)

                    last = (l == n_layers - 1)
                    if not last:
                        bnst = bnp.tile([h, NCH, 6], f32, tag="bnst",
                                        name=f"bnst_l{l + 1}")
                    h_out_f = h_out.bitcast(f32)
                    for g in range(NG):
                        psm = mps.tile([h, GW], f32, tag="psm", name=f"psm{l}_{g}")
                        if g % 4 == 0:
                            dummy_warm(mpd, f"mdum{l}_{g}")
                        nc.tensor.matmul(psm[:, 0:512], wf,
                                         h_in[:, g * GW:g * GW + 512],
                                         skip_group_check=True)
                        nc.tensor.matmul(psm[:, 512:GW], wf,
                                         h_in[:, g * GW + 512:(g + 1) * GW],
                                         skip_group_check=True)
                        sl = slice(g * GW, (g + 1) * GW)
                        nc.scalar.activation(h_out[:, sl], psm, AF.Relu,
                                             bias=bias2[:, 0:1])
                        if not last:
                            for c in range(GW // 512):
                                cc = g * (GW // 512) + c
                                nc.vector.bn_stats(bnst[:, cc, :],
                                                   h_out_f[:, cc * 512:(cc + 1) * 512])
                        else:
                            # head fused into layer 12's group loop
                            pso = mpso.tile([1, GW], f32, tag="pso",
                                            name=f"pso{g}")
                            nc.tensor.matmul(pso[:, 0:512], woutt_r,
                                             h_out[:, g * GW:g * GW + 512],
                                             skip_group_check=True)
                            nc.tensor.matmul(pso[:, 512:GW], woutt_r,
                                             h_out[:, g * GW + 512:(g + 1) * GW],
                                             skip_group_check=True)
                            nc.scalar.activation(out_row[:, sl], pso, AF.Identity,
                                                 bias=bout_sb[0:1, 0:1])
                    h_in, h_out = h_out, h_in

                nc.sync.dma_start(out_d.rearrange("b o -> o b"), out_row)
            bnp.release()
            hp.release()

    nc.compile()
    return nc


def make_in_maps(inputs, n_cores=N_CORES, b_local=B // N_CORES):
    """Host-side layout prep: shard+cast x to fp16 (both layouts)."""
    x = np.asarray(inputs["x"], np.float32)
    QD = D // 128
    xf = x.astype(F16)
    wint = np.ascontiguousarray(np.asarray(inputs["W_in"], np.float32).T)
    bin_ = np.asarray(inputs["b_in"], np.float32).reshape(-1, 1)
    growp = np.ascontiguousarray(
        np.asarray(inputs["bn_gamma_in"], np.float32).reshape(QD, 128).T)
    browp = np.ascontiguousarray(
        np.asarray(inputs["bn_beta_in"], np.float32).reshape(QD, 128).T)
    midwt = np.ascontiguousarray(
        np.asarray(inputs["mid_W"], np.float32).transpose(0, 2, 1))
    midg = np.ascontiguousarray(np.asarray(inputs["mid_gamma"], np.float32).T)
    midbeta = np.ascontiguousarray(np.asarray(inputs["mid_beta"], np.float32).T)
    midbias = np.ascontiguousarray(np.asarray(inputs["mid_b"], np.float32).T)
    woutt = np.ascontiguousarray(np.asarray(inputs["W_out"], np.float32).T)
    bout = np.asarray(inputs["b_out"], np.float32).reshape(1, 1)
    onesf = np.ones((128, 1), dtype=F16)
    identf2 = np.eye(2, dtype=np.float32)

    common = dict(wint=wint, bin=bin_, growp=growp, browp=browp, midwt=midwt,
                  midg=midg, midbeta=midbeta, midbias=midbias, woutt=woutt,
                  bout=bout, onesf=onesf, identf2=identf2)
    in_maps = []
    for c in range(n_cores):
        m = dict(common)
        shard = xf[c * b_local:(c + 1) * b_local]
        m["xb"] = np.ascontiguousarray(shard)
        m["xt"] = np.ascontiguousarray(shard.T)
        in_maps.append(m)
    return in_maps


@functools.lru_cache(maxsize=1)
def _get_program():
    return build_program()


def kernel(**inputs) -> np.ndarray:
    from concourse.bass_utils import run_bass_kernel_spmd
    nc = _get_program()
    in_maps = make_in_maps(inputs)
    res = run_bass_kernel_spmd(nc, in_maps, core_ids=list(range(N_CORES)))
    out = np.concatenate([res.results[c]["out"] for c in range(N_CORES)], axis=0)
    return out.astype(np.float32)


if __name__ == "__main__":
    nc = build_program(n_cores=2, b_local=2048, d=512, n_layers=2)
    print("built ok:", len(nc.inst_map), "instructions")
